# revision 1
# baseline (speedup 1.0000x reference)
"""Answer-pointer network forward pass on 8 TRN2 NeuronCores.

Data-parallel over batch: B=64 -> 8 batches per core, weights replicated.
No collectives; each core computes softmax attention maps (aP1, aP2) for
its batch shard and the host concatenates.

Host-side prep (free, outside HW exec): per-core shards are laid out so
every on-chip matmul has its contraction dim on SBUF partitions:
  - peT  [512,8,2048] fp16: passEnc transposed (feature-major) for the
    WPh linear (contract over d=512).
  - peN  [2048,8,512] fp16: passEnc natural (position-major) for the
    attention-weighted context ct (contract over p=2048).
Small weights/biases are packed host-side into per-partition contiguous
blobs: wpA (needed at kernel start), wpB (GRU weights, needed late),
wp32 (fp32 biases + identity), so each lands with one fast DMA.

Pointer-step softmaxes use raw exp without max subtraction (logits are
bounded by ||Vt2||_1 ~ 10, far below fp16/fp32 overflow), which lets the
step-1 softmax numerator weights w = exp(sP) be produced per 512-column
chunk and immediately consumed by the context reduction
ct = (w @ passEnc) / Z inside the P1 pipeline - no separate ct pass over
passEnc after the softmax.

Per-batch reductions (sP, sQ, rQ, ct) use masked stationary operands:
column b of the lhsT is kept, the rest zeroed, so the matmul for batch b
writes only PSUM row b; accumulating across b assembles the full [8, N]
result without partition-offset copies (compute engines cannot address
partitions at offsets other than 0/32/64/96).
"""

import numpy as np

try:
    import concourse.bass as bass
except ImportError:  # pragma: no cover
    import sys

    sys.path.insert(0, "/opt/trn_rl_repo")
    import concourse.bass as bass

import concourse.tile as tile
from concourse import bacc, mybir
from concourse.bass_utils import run_bass_kernel_spmd

F16 = mybir.dt.float16
F32 = mybir.dt.float32
AF = mybir.ActivationFunctionType
OP = mybir.AluOpType

H = 256      # hidden
D = 512      # 2*hidden
LP = 2048    # passage length
LQ = 64      # question length
B = 64       # global batch
BL = 8       # batch per core
G = 6 * H    # 1536, GRU gate width
NC = 8       # cores
NKT = D // 128   # 4 contraction tiles over d
NHT = H // 128   # 2 tiles over h
NPC = LP // 512  # 4 p-chunks of 512
NPT = LP // 128  # 16 p-tiles of 128


def _layout(entries):
    off, table = 0, {}
    for name, ln in entries:
        table[name] = (off, ln)
        off += ln
    return table, off


# wpA: needed from the start (Q phase + P1)
WA, WATOT = _layout([
    ("WQvT", NHT * H), ("WQuT", NHT * 2 * H), ("WPhT", NKT * H), ("WahT", NKT * H),
    ("VQrT", NHT), ("Vt1T", NHT * BL), ("Vt2T", NHT * BL), ("idh", 128),
    ("colm", BL * BL),
])
WQ, WQTOT = _layout([("qeT", NKT * BL * LQ), ("qeN", BL * D)])
# wpB: GRU weights, needed only mid-kernel
WB, WBTOT = _layout([("wihT", NKT * G), ("whhT", NKT * G), ("bgru", 2 * G)])
W32, W32TOT = _layout([("idf", 128), ("cqb", NHT), ("wb", NHT)])

_CACHED_NC = None


def _build():
    nc = bacc.Bacc("TRN2", target_bir_lowering=False, debug=False, num_devices=NC)

    peC = nc.dram_tensor("peC", [BL, NPC, 128, NKT, 512], F16, kind="ExternalInput").ap()
    peN = nc.dram_tensor("peN", [LP, BL, D], F16, kind="ExternalInput").ap()
    wpA = nc.dram_tensor("wpA", [128, WATOT], F16, kind="ExternalInput").ap()
    wpQ = nc.dram_tensor("wpQ", [128, WQTOT], F16, kind="ExternalInput").ap()
    wpB = nc.dram_tensor("wpB", [128, WBTOT], F16, kind="ExternalInput").ap()
    wp32 = nc.dram_tensor("wp32", [128, W32TOT], F32, kind="ExternalInput").ap()
    out = nc.dram_tensor("out", [2, BL, LP], F32, kind="ExternalOutput").ap()

    with tile.TileContext(nc) as tc:
        sing = tc.alloc_tile_pool(name="sing", bufs=1)

        def _single(shape, dtype, name):
            return sing.tile(shape, dtype, name=name, tag=name)

        chunkp = tc.alloc_tile_pool(name="chunk", bufs=6)
        t2p = tc.alloc_tile_pool(name="t2", bufs=3)
        rowp = tc.alloc_tile_pool(name="rows", bufs=2)
        wmp = tc.alloc_tile_pool(name="wm", bufs=2)
        # PSUM budget: ppp 3 banks + rowps 4 + trp 1 = 8
        ppp = tc.alloc_tile_pool(name="ppp", bufs=3, space="PSUM")
        rowps = tc.alloc_tile_pool(name="rowps", bufs=4, space="PSUM")
        trp = tc.alloc_tile_pool(name="trp", bufs=1, space="PSUM")

        # ---- packed weights ----
        wpA_s = _single([128, WATOT], F16, "wpA_s")
        nc.scalar.dma_start(wpA_s, wpA)
        wp32_s = _single([128, W32TOT], F32, "wp32_s")
        nc.scalar.dma_start(wp32_s, wp32)
        wpQ_s = _single([128, WQTOT], F16, "wpQ_s")
        nc.scalar.dma_start(wpQ_s, wpQ)
        wpB_s = _single([128, WBTOT], F16, "wpB_s")
        nc.scalar.dma_start(wpB_s, wpB)

        def sA(name):
            o, ln = WA[name]
            return wpA_s[:, o:o + ln]

        WQvT_s = sA("WQvT").rearrange("p (kt h) -> p kt h", kt=NHT)
        WQuT_s = sA("WQuT").rearrange("p (kt h) -> p kt h", kt=NKT)
        WPhT_s = sA("WPhT").rearrange("p (kt h) -> p kt h", kt=NKT)
        WahT_s = sA("WahT").rearrange("p (kt h) -> p kt h", kt=NKT)
        VQrT_s = sA("VQrT").rearrange("p (ht o) -> p ht o", ht=NHT)
        Vt1T_s = sA("Vt1T").rearrange("p (ht b) -> p ht b", ht=NHT)
        Vt2T_s = sA("Vt2T").rearrange("p (ht b) -> p ht b", ht=NHT)
        idh_s = sA("idh")
        colm_s = sA("colm").rearrange("p (b c) -> p b c", b=BL)
        qeT_s = wpQ_s[:, WQ["qeT"][0]:WQ["qeT"][0] + NKT * BL * LQ].rearrange(
            "p (kt bq) -> p kt bq", kt=NKT)
        qeN_s = wpQ_s[0:LQ, WQ["qeN"][0]:WQ["qeN"][0] + BL * D]
        wihT_s = wpB_s[:, WB["wihT"][0]:WB["wihT"][0] + NKT * G].rearrange(
            "p (kt g) -> p kt g", kt=NKT)
        whhT_s = wpB_s[:, WB["whhT"][0]:WB["whhT"][0] + NKT * G].rearrange(
            "p (kt g) -> p kt g", kt=NKT)
        bgru_s = wpB_s[0:BL, WB["bgru"][0]:WB["bgru"][0] + 2 * G].rearrange(
            "b (two g) -> b two g", two=2)
        idf_s = wp32_s[:, W32["idf"][0]:W32["idf"][0] + 128]
        cqb_s = wp32_s[:, W32["cqb"][0]:W32["cqb"][0] + NHT]
        wb_s = wp32_s[:, W32["wb"][0]:W32["wb"][0] + NHT].rearrange(
            "p (ht o) -> p ht o", ht=NHT)

        # persistent activations
        ppr_s = _single([128, NHT, BL, NPC, 512], F16, "ppr_s")  # raw passP
        biasP_s = _single([128, 2, NHT, BL], F32, "biasP_s")
        rq1_s = _single([BL, D], F32, "rq1_s")
        ct_s = _single([BL, D], F32, "ct_s")
        rq2_s = _single([BL, D], F32, "rq2_s")
        rq1T_s = [_single([128, BL], F16, f"rq1T{k}") for k in range(NKT)]
        ctT_s = [_single([128, BL], F16, f"ctT{k}") for k in range(NKT)]
        rq2T_s = [_single([128, BL], F16, f"rq2T{k}") for k in range(NKT)]

        # masked per-batch stationary operands (column b kept, rest zero)
        vt1m, vt2m = [], []
        for b in range(BL):
            m1 = _single([128, NHT, BL], F16, f"vt1m{b}")
            nc.vector.memset(m1, 0.0)
            nc.vector.tensor_copy(m1[:, :, b:b + 1], Vt1T_s[:, :, b:b + 1])
            vt1m.append(m1)
            m2 = _single([128, NHT, BL], F16, f"vt2m{b}")
            nc.vector.memset(m2, 0.0)
            nc.vector.tensor_copy(m2[:, :, b:b + 1], Vt2T_s[:, :, b:b + 1])
            vt2m.append(m2)

        def bcast_dim(ap, axis, size):
            """Insert a stride-0 (broadcast) free dim at position axis."""
            entries = list(ap.ap)
            entries.insert(axis, [0, size])
            return bass.AP(tensor=ap.tensor, offset=ap.offset, ap=entries)

        def rows_to_colsT(src_rows, dst_tiles):
            """src [8, 512] f32 -> four [128, 8] f16 tiles (per-kt, so
            consumers of slice kt start as soon as that slice lands)."""
            for kt in range(NKT):
                ps_t = ppp.tile([128, BL], F32, tag="acc", name="ps_t")
                nc.tensor.transpose(ps_t, src_rows[:, kt * 128:(kt + 1) * 128],
                                    idf_s[:BL, :BL])
                nc.vector.tensor_copy(dst_tiles[kt], ps_t)

        def wah_bias(rqT, st):
            """biasP[:, st, ht, :] = WahT.T @ rqT + (WPh_b + Wah_b)."""
            for ht in range(NHT):
                ps_w = ppp.tile([128, BL], F32, tag="acc", name="ps_w")
                for kt in range(NKT):
                    nc.tensor.matmul(ps_w, lhsT=WahT_s[:, kt, ht * 128:(ht + 1) * 128],
                                     rhs=rqT[kt], start=kt == 0, stop=kt == NKT - 1)
                nc.vector.tensor_scalar(biasP_s[:, st, ht, :], ps_w, wb_s[:, ht, :],
                                        None, op0=OP.add)

        # ========== P1: passP + tanh + sP1 + online exp/ct ==========
        # Software-pipelined emission: A(pc)=passP matmuls (no Q dependency),
        # B(pc)=tanh+sP+exp (needs Q's biasP), C(pc)=ct reduction (needs B).
        # Order A0 A1 B0 A2 B1 C0 A3 B2 C1 B3 C2 C3 keeps every stage's
        # inputs at least one full PE-queue block ahead -> no sequencer
        # head-of-line stalls.
        w1_s = rowp.tile([BL, LP], F16, tag="bigrow", name="w1_s")  # exp(sP1)
        zpart = _single([BL, NPC], F32, "zpart")
        ps_ct = rowps.tile([BL, D], F32, tag="row", name="ps_ct")
        ps_sp1 = {}
        t2saved = {}

        def em_a(pc, b):
            petc = chunkp.tile([128, NKT, 512], F16, tag="pe", name="petc")
            nc.sync.dma_start(petc, peC[b, pc])
            ps_pps = [ppp.tile([128, 512], F32, tag="acc", name=f"ps_pp{ht}")
                      for ht in range(NHT)]
            for kt in range(NKT):
                for ht in range(NHT):
                    nc.tensor.matmul(ps_pps[ht],
                                     lhsT=WPhT_s[:, kt, ht * 128:(ht + 1) * 128],
                                     rhs=petc[:, kt, :],
                                     start=kt == 0, stop=kt == NKT - 1)
            for ht in range(NHT):
                nc.vector.tensor_copy(ppr_s[:, ht, b, pc, :], ps_pps[ht])

        def em_btanh(pc, b):
            t2 = t2p.tile([128, NHT, 512], F16, tag="t2", name="t2a")
            for ht in range(NHT):
                nc.scalar.activation(t2[:, ht, :], ppr_s[:, ht, b, pc, :],
                                     AF.Tanh, bias=biasP_s[:, 0, ht, b:b + 1],
                                     scale=1.0)
            t2saved[(pc, b)] = t2

        def em_bsp(pc, b):
            t2 = t2saved.pop((pc, b))
            for ht in range(NHT):
                nc.tensor.matmul(ps_sp1[pc], lhsT=vt2m[b][:, ht, :],
                                 rhs=t2[:, ht, :],
                                 start=(b == 0 and ht == 0),
                                 stop=(b == BL - 1 and ht == NHT - 1))

        def stage_ab(apc, bpc):
            if bpc is not None:
                ps_sp1[bpc] = rowps.tile([BL, 512], F32, tag="row",
                                         name=f"ps_sp1_{bpc}")
            prev = None
            for b in range(BL):
                if apc is not None:
                    em_a(apc, b)
                if bpc is not None:
                    em_btanh(bpc, b)
                    if prev is not None:
                        em_bsp(bpc, prev)
                    prev = b
            if bpc is not None:
                em_bsp(bpc, prev)
                nc.scalar.activation(w1_s[:, bpc * 512:(bpc + 1) * 512],
                                     ps_sp1[bpc], AF.Exp,
                                     accum_out=zpart[:, bpc:bpc + 1])

        def stage_c(pc):
            wmall = wmp.tile([128, 4, BL, BL], F16, tag="wm", name="wmall")
            for j in range(4):
                ps_wt = trp.tile([128, BL], F16, tag="tr", name="ps_wt")
                nc.tensor.transpose(
                    ps_wt, w1_s[:, pc * 512 + j * 128: pc * 512 + (j + 1) * 128],
                    idh_s[:BL, :BL])
                nc.vector.tensor_mul(wmall[:, j, :, :],
                                     bcast_dim(ps_wt[:, :], 1, BL),
                                     colm_s[:, :, :])
            for j in range(4):
                for bh in range(2):
                    penb = chunkp.tile([128, 4, 512], F16, tag="pe", name="penb")
                    pt = pc * 4 + j
                    nc.scalar.dma_start(
                        penb, peN[pt * 128:(pt + 1) * 128, bh * 4:(bh + 1) * 4, :])
                    for bi in range(4):
                        b = bh * 4 + bi
                        nc.tensor.matmul(
                            ps_ct, lhsT=wmall[:, j, b, :], rhs=penb[:, bi, :],
                            start=(pc == 0 and j == 0 and bh == 0 and bi == 0),
                            stop=(pc == NPC - 1 and j == 3 and bh == 1 and bi == 3))

        gi_s = _single([BL, G], F16, "gi_s")

        def q_phase():
            # ================= Q phase =================
            ps_qv = trp.tile([128, NHT], F32, tag="tr", name="ps_qv")
            for ht in range(NHT):
                for kt in range(NHT):
                    nc.tensor.matmul(ps_qv[:, ht:ht + 1],
                                     lhsT=WQvT_s[:, kt, ht * 128:(ht + 1) * 128],
                                     rhs=VQrT_s[:, kt, :], start=kt == 0, stop=kt == NHT - 1)
            cb_s = _single([128, NHT], F32, "cb_s")
            nc.vector.tensor_add(cb_s, ps_qv, cqb_s)

            tqT_s = _single([128, NHT, BL * LQ], F16, "tqT_s")
            for ht in range(NHT):
                ps_tq = ppp.tile([128, 512], F32, tag="acc", name="ps_tq")
                for kt in range(NKT):
                    nc.tensor.matmul(ps_tq, lhsT=WQuT_s[:, kt, ht * 128:(ht + 1) * 128],
                                     rhs=qeT_s[:, kt, :], start=kt == 0, stop=kt == NKT - 1)
                nc.scalar.activation(tqT_s[:, ht, :], ps_tq, AF.Tanh,
                                     bias=cb_s[:, ht:ht + 1], scale=1.0)

            # sQ assembled via masked lhsT accumulation: [8, 64]
            ps_sq = rowps.tile([BL, LQ], F32, tag="row", name="ps_sq")
            for b in range(BL):
                for ht in range(NHT):
                    nc.tensor.matmul(ps_sq, lhsT=vt1m[b][:, ht, :],
                                     rhs=tqT_s[:, ht, b * LQ:(b + 1) * LQ],
                                     start=(b == 0 and ht == 0),
                                     stop=(b == BL - 1 and ht == NHT - 1))
            esq = _single([BL, LQ], F32, "esq")
            zq = _single([BL, 1], F32, "zq")
            nc.scalar.activation(esq, ps_sq, AF.Exp, accum_out=zq)
            rzq = _single([BL, 1], F32, "rzq")
            nc.vector.reciprocal(rzq, zq)
            a_s = _single([BL, LQ], F16, "a_s")
            nc.vector.tensor_scalar(a_s, esq, rzq, None, op0=OP.mult)

            ps_at = trp.tile([LQ, BL], F16, tag="tr", name="ps_at")
            nc.tensor.transpose(ps_at, a_s, idh_s[:BL, :BL])
            atm_s = _single([LQ, BL, BL], F16, "atm_s")
            nc.vector.tensor_mul(atm_s,
                                 bcast_dim(ps_at[:, :], 1, BL),
                                 colm_s[0:LQ, :, :])
            ps_rq = rowps.tile([BL, D], F32, tag="row", name="ps_rq")
            for b in range(BL):
                nc.tensor.matmul(ps_rq, lhsT=atm_s[:, b, :],
                                 rhs=qeN_s[:, b * D:(b + 1) * D],
                                 start=b == 0, stop=b == BL - 1)
            nc.vector.tensor_copy(rq1_s, ps_rq)

            rows_to_colsT(rq1_s, rq1T_s)
            wah_bias(rq1T_s, 0)

            # gi = rq1 @ wih.T + bih: only needs Q results; computed here so the
            # GRU join later only waits on the ct-dependent half
            for nch in range(G // 512):
                ps_gi = rowps.tile([BL, 512], F32, tag="row", name="ps_gi")
                for kt in range(NKT):
                    nc.tensor.matmul(ps_gi, lhsT=rq1T_s[kt],
                                     rhs=wihT_s[:, kt, nch * 512:(nch + 1) * 512],
                                     start=kt == 0, stop=kt == NKT - 1)
                nc.vector.tensor_add(gi_s[:, nch * 512:(nch + 1) * 512], ps_gi,
                                     bgru_s[:, 0, nch * 512:(nch + 1) * 512])


        stage_ab(0, None)
        q_phase()
        stage_ab(1, 0)
        stage_ab(2, 1)
        stage_c(0)
        stage_ab(3, 2)
        stage_c(1)
        stage_c(2)
        stage_ab(None, 3)
        stage_c(3)

        z1_s = _single([BL, 1], F32, "z1_s")
        nc.vector.reduce_sum(z1_s, zpart, axis=mybir.AxisListType.X)
        rz1_s = _single([BL, 1], F32, "rz1_s")
        nc.vector.reciprocal(rz1_s, z1_s)
        ap1_s = rowp.tile([BL, LP], F32, tag="bigrow", name="ap1_s")
        nc.vector.tensor_scalar(ap1_s, w1_s, rz1_s, None, op0=OP.mult)
        nc.scalar.dma_start(out=out[0], in_=ap1_s)
        nc.vector.tensor_scalar(ct_s, ps_ct, rz1_s, None, op0=OP.mult)

        # ================= GRU cell =================
        rows_to_colsT(ct_s, ctT_s)
        gh_s = _single([BL, G], F16, "gh_s")
        for nch in range(G // 512):
            ps_g = rowps.tile([BL, 512], F32, tag="row", name="ps_g")
            for kt in range(NKT):
                nc.tensor.matmul(ps_g, lhsT=ctT_s[kt],
                                 rhs=whhT_s[:, kt, nch * 512:(nch + 1) * 512],
                                 start=kt == 0, stop=kt == NKT - 1)
            nc.vector.tensor_add(gh_s[:, nch * 512:(nch + 1) * 512], ps_g,
                                 bgru_s[:, 1, nch * 512:(nch + 1) * 512])
        rzin_s = _single([BL, 2 * D], F16, "rzin_s")
        nc.vector.tensor_add(rzin_s, gi_s[:, 0:2 * D], gh_s[:, 0:2 * D])
        rz_s = _single([BL, 2 * D], F16, "rz_s")
        nc.scalar.activation(rz_s, rzin_s, AF.Sigmoid)
        nin_s = _single([BL, D], F32, "nin_s")
        nc.vector.tensor_mul(nin_s, rz_s[:, 0:D], gh_s[:, 2 * D:3 * D])
        nin2_s = _single([BL, D], F32, "nin2_s")
        nc.vector.tensor_add(nin2_s, nin_s, gi_s[:, 2 * D:3 * D])
        n_s = _single([BL, D], F32, "n_s")
        nc.scalar.activation(n_s, nin2_s, AF.Tanh)
        # h' = n + z*(ct - n)
        d1_s = _single([BL, D], F32, "d1_s")
        nc.vector.tensor_sub(d1_s, ct_s, n_s)
        nc.vector.tensor_mul(d1_s, d1_s, rz_s[:, D:2 * D])
        nc.vector.tensor_add(rq2_s, n_s, d1_s)

        rows_to_colsT(rq2_s, rq2T_s)
        wah_bias(rq2T_s, 1)

        # ========== P2: tanh + sP2 (passP reused), raw exp ==========
        ps_sp2 = [rowps.tile([BL, 512], F32, tag="row", name=f"ps_sp2_{pc}")
                  for pc in range(NPC)]
        for b in range(BL):
            t2b = t2p.tile([128, NHT, LP], F16, tag="t2", name="t2b")
            for ht in range(NHT):
                nc.scalar.activation(t2b[:, ht, :], ppr_s[:, ht, b, :, :], AF.Tanh,
                                     bias=biasP_s[:, 1, ht, b:b + 1], scale=1.0)
            for pc in range(NPC):
                for ht in range(NHT):
                    nc.tensor.matmul(ps_sp2[pc],
                                     lhsT=vt2m[b][:, ht, :],
                                     rhs=t2b[:, ht, pc * 512:(pc + 1) * 512],
                                     start=(b == 0 and ht == 0),
                                     stop=(b == BL - 1 and ht == NHT - 1))
        w2_s = rowp.tile([BL, LP], F16, tag="bigrow", name="w2_s")
        zp2 = _single([BL, NPC], F32, "zp2")
        for pc in range(NPC):
            nc.scalar.activation(w2_s[:, pc * 512:(pc + 1) * 512], ps_sp2[pc], AF.Exp,
                                 accum_out=zp2[:, pc:pc + 1])
        z2_s = _single([BL, 1], F32, "z2_s")
        nc.vector.reduce_sum(z2_s, zp2, axis=mybir.AxisListType.X)
        rz2_s = _single([BL, 1], F32, "rz2_s")
        nc.vector.reciprocal(rz2_s, z2_s)
        ap2_s = rowp.tile([BL, LP], F32, tag="bigrow", name="ap2_s")
        for pc in range(NPC):
            nc.vector.tensor_scalar(ap2_s[:, pc * 512:(pc + 1) * 512],
                                    w2_s[:, pc * 512:(pc + 1) * 512], rz2_s,
                                    None, op0=OP.mult)
            nc.scalar.dma_start(out=out[1, :, pc * 512:(pc + 1) * 512],
                                in_=ap2_s[:, pc * 512:(pc + 1) * 512])

        trp.release()
        rowps.release()
        ppp.release()
        wmp.release()
        rowp.release()
        t2p.release()
        chunkp.release()
        sing.release()

    nc.compile()
    return nc


def _get_nc():
    global _CACHED_NC
    if _CACHED_NC is None:
        _CACHED_NC = _build()
    return _CACHED_NC


def _tiles(mat, nkt):  # [nkt*128, X] -> [128, nkt*X]
    x = mat.shape[1]
    return np.ascontiguousarray(
        mat.reshape(nkt, 128, x).transpose(1, 0, 2).reshape(128, nkt * x))


def _packA(f, Vt1, Vt2):
    wp = np.zeros((128, WATOT), dtype=np.float16)

    def put(name, arr):
        o, ln = WA[name]
        assert arr.shape[1] == ln, (name, arr.shape, ln)
        wp[:arr.shape[0], o:o + ln] = arr

    put("WQvT", _tiles(f["WQv_W"].T.astype(np.float16), NHT))
    put("WQuT", _tiles(f["WQu_W"].T.astype(np.float16), NKT))
    put("WPhT", _tiles(f["WPh_W"].T.astype(np.float16), NKT))
    put("WahT", _tiles(f["Wah_W"].T.astype(np.float16), NKT))
    put("VQrT", _tiles(f["VQr"].reshape(1, H).T.astype(np.float16), NHT))
    put("Vt1T", _tiles(Vt1.astype(np.float16), NHT))
    put("Vt2T", _tiles(Vt2.astype(np.float16), NHT))
    put("idh", np.eye(128, dtype=np.float16))
    put("colm", np.broadcast_to(np.eye(BL, dtype=np.float16).reshape(1, BL * BL),
                                (128, BL * BL)))
    return wp


def _packQ(qe):
    wp = np.zeros((128, WQTOT), dtype=np.float16)
    o, ln = WQ["qeT"]
    qeT = np.ascontiguousarray(qe.transpose(2, 1, 0)).astype(np.float16)
    wp[:, o:o + ln] = _tiles(qeT.reshape(D, BL * LQ), NKT)
    o, ln = WQ["qeN"]
    wp[:LQ, o:o + ln] = qe.astype(np.float16).reshape(LQ, BL * D)
    return wp


def _packB(f):
    wp = np.zeros((128, WBTOT), dtype=np.float16)
    o, ln = WB["wihT"]
    wp[:, o:o + ln] = _tiles(f["gru_wih"].T.astype(np.float16), NKT)
    o, ln = WB["whhT"]
    wp[:, o:o + ln] = _tiles(f["gru_whh"].T.astype(np.float16), NKT)
    o, ln = WB["bgru"]
    bg = np.stack([np.broadcast_to(f["gru_bih"], (BL, G)),
                   np.broadcast_to(f["gru_bhh"], (BL, G))],
                  axis=1).astype(np.float16).reshape(BL, 2 * G)
    wp[:BL, o:o + ln] = bg
    return wp


def _pack32(f):
    wp = np.zeros((128, W32TOT), dtype=np.float32)
    o, ln = W32["idf"]
    wp[:, o:o + ln] = np.eye(128, dtype=np.float32)
    o, ln = W32["cqb"]
    wp[:, o:o + ln] = (f["WQu_b"] + f["WQv_b"]).astype(np.float32).reshape(NHT, 128).T
    o, ln = W32["wb"]
    wp[:, o:o + ln] = (f["WPh_b"] + f["Wah_b"]).astype(np.float32).reshape(NHT, 128).T
    return wp


def make_in_maps(f):
    passEnc, quesEnc = f["passEnc"], f["quesEnc"]
    wp32 = _pack32(f)
    wpB = _packB(f)
    in_maps = []
    for i in range(NC):
        s = slice(i * BL, (i + 1) * BL)
        pe = passEnc[:, s, :]
        qe = quesEnc[:, s, :]
        wpA = _packA(f, f["Vt1"][s, :, 0].T, f["Vt2"][s, :, 0].T)
        wpQ_ = _packQ(qe)
        peC = np.ascontiguousarray(
            pe.astype(np.float16).reshape(NPC, 512, BL, NKT, 128).transpose(
                2, 0, 4, 3, 1))
        in_maps.append({
            "peC": peC,
            "peN": pe.astype(np.float16),
            "wpA": wpA, "wpQ": wpQ_, "wpB": wpB, "wp32": wp32,
        })
    return in_maps


def kernel(**inputs):
    f = {k: np.asarray(v) for k, v in inputs.items()}
    in_maps = make_in_maps(f)
    nc = _get_nc()
    res = run_bass_kernel_spmd(nc, in_maps, core_ids=list(range(NC)))
    aP1 = np.concatenate([res.results[i]["out"][0] for i in range(NC)], axis=0)
    aP2 = np.concatenate([res.results[i]["out"][1] for i in range(NC)], axis=0)
    return (aP1.astype(np.float32), aP2.astype(np.float32))

